# revision 1
# baseline (speedup 1.0000x reference)
"""ExemplarAttention Trainium2 kernel (8 NeuronCores, batch-sharded).

logits[b,c] = gamma * log(sum_{n:label[n]=c} exp(-beta * sum_k w_k (x[b,k]-e[n,k])^2) + eps)

Strategy:
  - Data-parallel over batch B=1024 across 8 cores (128 rows/core = one partition tile).
  - Host precomputes the tiny constrained params (softmax(w), beta, gamma),
    x^2@w (per-row bias), e^2@w, and sorts exemplars by class label so the
    per-class scatter-add becomes contiguous segment sums.
  - On device, per 2048-column PSUM super-tile:
      psum = (ones x -S*e2w/2)            [K=1 bf16 matmul, start=True]
           + S * sum_k xw_t[k].T @ e_t[k] [fp8 DoubleRow matmuls, 2 groups x K=256]
    i.e. psum[m,n] = S * (cross[m,n] - e2w[n]/2).   (S rescales x*w into fp8 range)
  - ScalarE: exp((2*beta/S)*psum + (-beta*x2w)[m]) per class-segment piece with
    accum_out -> per-class partial sums directly (no one-hot GEMM, no transpose).
  - Tail: one 3D tensor_reduce merges the piece partials, Ln(+1e-9), *gamma, DMA out.
"""

import os
from contextlib import ExitStack

import numpy as np

B, N, D, C = 1024, 16384, 512, 10
NCORES = 8
B_LOC = B // NCORES          # 128
NG = 2                       # DoubleRow groups (K=256 each)
SUPER = 2048                 # psum super-tile width (4 banks)
NSUPER = N // SUPER
NTILE = 512                  # matmul free dim (1 psum bank)
EPS = 1e-9
S_SCALE = 128.0              # fp8 scale applied to x*w (and the e2w aug row)

# e_t DMA blocks: (col_start, width), two supers each. Coarse blocks keep the
# number of PE wait-points low (frequent micro-waits make the PE's HAM clock
# gate oscillate between 1.2 and 2.4 GHz, halving matmul throughput).
ET_BLOCKS = [(c, 2 * SUPER) for c in range(0, N, 2 * SUPER)]
# PE warmup matmuls issued before the main stream: they keep the PE busy for
# the HAM SHORT window (~3.4us) while giving the e_t DMA stream a head start
# so the matmul stream never catches the DMA stream (which would micro-stall
# the PE and re-throttle the clock gate).
N_WARMUP_MM = 14

_prog_cache = {}


def _np_dt(mybir, name):
    return mybir.dt.np(getattr(mybir.dt, name))


def _compute_pieces(counts):
    """Split each class's sorted-exemplar segment at SUPER boundaries.

    Returns (pieces, maxp): pieces is a list of (super_idx, cls, piece_idx,
    g0, g1) with global column range [g0, g1)."""
    starts = np.concatenate([[0], np.cumsum(counts)]).astype(int)
    pieces = []
    piece_counter = [0] * C
    for c in range(C):
        g0, g1 = int(starts[c]), int(starts[c + 1])
        while g0 < g1:
            end = min(g1, (g0 // SUPER + 1) * SUPER)
            pieces.append((g0 // SUPER, c, piece_counter[c], g0, end))
            piece_counter[c] += 1
            g0 = end
    maxp = max(piece_counter) if max(piece_counter) > 0 else 1
    return pieces, maxp


def _build_program(pieces, maxp, beta, gamma):
    import concourse.bass as bass  # noqa: F401
    import concourse.tile as tile
    from concourse import bacc, mybir

    fp8 = mybir.dt.float8e4
    bf16 = mybir.dt.bfloat16
    f32 = mybir.dt.float32

    nc = bacc.Bacc("TRN2", target_bir_lowering=False, debug=False,
                   num_devices=NCORES)

    e_t_d = nc.dram_tensor("e_t", [NG, 128, 2, N], fp8, kind="ExternalInput").ap()
    xw_t_d = nc.dram_tensor("xw_t", [128, NG, 2, B_LOC], fp8,
                            kind="ExternalInput").ap()
    aug_d = nc.dram_tensor("aug", [1, N + 128], bf16, kind="ExternalInput").ap()
    bias_d = nc.dram_tensor("bias", [B_LOC, 1], f32, kind="ExternalInput").ap()
    out_d = nc.dram_tensor("logits", [B_LOC, C], f32, kind="ExternalOutput").ap()

    act_scale = float(2.0 * beta / S_SCALE)

    by_super = [[] for _ in range(NSUPER)]
    for s, c, p, g0, g1 in pieces:
        by_super[s].append((c, p, g0, g1))

    # super -> (block index, col offset within block)
    sup_block = {}
    for bi, (c0, w) in enumerate(ET_BLOCKS):
        for s in range(c0 // SUPER, (c0 + w) // SUPER):
            sup_block[s] = (bi, s * SUPER - c0)

    with tile.TileContext(nc) as tc, ExitStack() as ctx:
        singles = ctx.enter_context(tc.tile_pool(name="singles", bufs=1))
        et_pool = ctx.enter_context(tc.tile_pool(name="et", bufs=len(ET_BLOCKS) * NG))
        psum_pool = ctx.enter_context(tc.tile_pool(name="ps", bufs=2, space="PSUM"))
        sc_pool = ctx.enter_context(tc.tile_pool(name="sc", bufs=2))

        # Dummy activation first so the ACT table load runs during the DMA
        # startup window instead of blocking the first real exp.
        dummy = singles.tile([128, 1], f32)
        nc.vector.memset(dummy[:, :], 0.0)
        nc.scalar.activation(out=dummy[:, :], in_=dummy[:, :],
                             func=mybir.ActivationFunctionType.Exp, scale=1.0)

        # aug row (-S*e2w/2) + ones row for the K=1 psum pre-fill matmuls.
        aug_sb = singles.tile([1, N + 128], bf16)
        nc.sync.dma_start(out=aug_sb[:, :], in_=aug_d[:, :])
        bias_sb = singles.tile([B_LOC, 1], f32)
        nc.scalar.dma_start(out=bias_sb[:, :], in_=bias_d[:, :])

        et_tiles = {}
        dma_engines = [nc.sync, nc.scalar]
        di = 0
        for bi, (c0, w) in enumerate(ET_BLOCKS):
            for g in range(NG):
                et_tiles[(bi, g)] = et_pool.tile(
                    [128, 2, 2 * SUPER], fp8, tag="et", name=f"et{bi}_{g}")

        def load_et(bi, g, eng=None):
            nonlocal di
            c0, w = ET_BLOCKS[bi]
            (eng or dma_engines[di % len(dma_engines)]).dma_start(
                out=et_tiles[(bi, g)][:, :, :w], in_=e_t_d[g, :, :, c0:c0 + w])
            di += 1

        # Block 0 rides at the head of both rings so super 0's matmuls can
        # start as soon as possible.
        load_et(0, 0, nc.sync)
        load_et(0, 1, nc.scalar)

        def emit_aug(s, ps):
            for j in range(SUPER // NTILE):
                cs = slice(j * NTILE, (j + 1) * NTILE)
                gcs = slice(s * SUPER + j * NTILE, s * SUPER + (j + 1) * NTILE)
                nc.tensor.matmul(ps[:, cs], lhsT=aug_sb[:, N:N + B_LOC],
                                 rhs=aug_sb[:, gcs], start=True, stop=False)

        # Warmup + hoisted aug matmuls for supers 0/1: they only need aug_sb,
        # so they run during the e_t DMA window — prefilling PSUM, warming
        # the PE clock gate (HAM), and giving the DMA stream a head start.
        ps_pre = [psum_pool.tile([128, SUPER], f32, tag="ps", name=f"ps{s}")
                  for s in range(2)]
        # Warmup operands come from a memset tile so the warmup matmuls have
        # no DMA dependency: full-array (K=128) PE activity starts right
        # after the preamble, opens the HAM clock gate, and intentionally
        # delays the main stream until the e_t DMA has an uncatchable lead
        # (a main stream that catches the DMA micro-stalls and re-throttles
        # the PE clock to 1.2 GHz).
        dmy = singles.tile([128, B_LOC + NTILE], bf16)
        nc.vector.memset(dmy[:, :], 0.0)
        for _ in range(N_WARMUP_MM):
            nc.tensor.matmul(ps_pre[0][:, 0:NTILE], lhsT=dmy[:, 0:B_LOC],
                             rhs=dmy[:, B_LOC:], start=True, stop=True)
        for s in (0, 1):
            emit_aug(s, ps_pre[s])

        # x*w weights (tiny) ride the scalar ring behind bias.
        xw_sb = singles.tile([128, NG, 2, B_LOC], fp8)
        nc.scalar.dma_start(out=xw_sb[:, :, :, :], in_=xw_t_d[:, :, :, :])

        acc = singles.tile([128, C * maxp], f32)
        nc.vector.memset(acc[:, :], 0.0)
        eps_sb = singles.tile([128, 1], f32)
        nc.vector.memset(eps_sb[:, :], float(EPS))

        for bi in range(1, len(ET_BLOCKS)):
            for g in range(NG):
                load_et(bi, g)

        for s in range(NSUPER):
            bi, off = sup_block[s]
            if s < 2:
                ps = ps_pre[s]
            else:
                ps = psum_pool.tile([128, SUPER], f32, tag="ps", name=f"ps{s}")
                emit_aug(s, ps)
            # DoubleRow main matmuls, k-major so weights reload once per group
            for g in range(NG):
                et = et_tiles[(bi, g)]
                for j in range(SUPER // NTILE):
                    cs = slice(j * NTILE, (j + 1) * NTILE)
                    ecs = slice(off + j * NTILE, off + (j + 1) * NTILE)
                    nc.tensor.matmul(
                        ps[:, cs], lhsT=xw_sb[:, g, :, :],
                        rhs=et[:, :, ecs], start=False, stop=(g == NG - 1),
                        perf_mode=mybir.MatmulPerfMode.DoubleRow)

            # One wide exp per super on ScalarE; the per-class segment sums
            # run on the otherwise-idle VectorE from the f32 scratch.
            sc = sc_pool.tile([128, SUPER], f32, tag="sc")
            nc.scalar.activation(
                out=sc[:, :],
                in_=ps[:, :],
                func=mybir.ActivationFunctionType.Exp,
                bias=bias_sb[:, :],
                scale=act_scale,
            )
            for c, p, g0, g1 in by_super[s]:
                l0, l1 = g0 - s * SUPER, g1 - s * SUPER
                nc.vector.tensor_reduce(
                    out=acc[:, c * maxp + p:c * maxp + p + 1],
                    in_=sc[:, l0:l1],
                    axis=mybir.AxisListType.X,
                    op=mybir.AluOpType.add,
                )

        class_sum = singles.tile([128, C], f32)
        nc.vector.tensor_reduce(
            out=class_sum[:, :],
            in_=acc.rearrange("q (c m) -> q c m", c=C),
            axis=mybir.AxisListType.X,
            op=mybir.AluOpType.add,
        )
        logits_sb = singles.tile([128, C], f32)
        nc.scalar.activation(
            out=logits_sb[:, :],
            in_=class_sum[:, :],
            func=mybir.ActivationFunctionType.Ln,
            bias=eps_sb[:, :],
            scale=1.0,
        )
        nc.vector.tensor_scalar_mul(logits_sb[:, :], logits_sb[:, :], float(gamma))
        nc.sync.dma_start(out=out_d[:, :], in_=logits_sb[:, :])

    nc.compile()

    # Both Exp and Ln live in act-func-set 6 (natural_log_exp_and_others);
    # the insertion pass picks per-func sets and pays a mid-kernel reload.
    # Point the first load at set 6 and drop the now-redundant extras.
    loads = [(b, i) for b in nc.main_func.blocks for i in b.instructions
             if isinstance(i, mybir.InstLoadActFuncSet)]
    if loads:
        loads[0][1].act_func_set_id = 6
        for b, i in loads[1:]:
            if i.sync_info is None or (
                    not i.sync_info.on_wait and not i.sync_info.on_update):
                b.instructions.remove(i)
            else:
                i.act_func_set_id = 6
    return nc


def _prepare(x, ex_feats, ex_labels, w_unconstrained, gamma_unconstrained,
             beta_unconstrained):
    from concourse import mybir

    x = np.asarray(x, dtype=np.float64)
    e = np.asarray(ex_feats, dtype=np.float64)
    labels = np.asarray(ex_labels).astype(np.int64)
    wu = np.asarray(w_unconstrained, dtype=np.float64)

    beta = float(np.log1p(np.exp(np.float64(beta_unconstrained)))) + EPS
    gamma = float(np.log1p(np.exp(np.float64(gamma_unconstrained)))) + EPS
    wexp = np.exp(wu - wu.max())
    w = wexp / wexp.sum() + EPS

    perm = np.argsort(labels, kind="stable")
    e_sorted = e[perm]
    counts = np.bincount(labels[perm], minlength=C)

    bf16 = _np_dt(mybir, "bfloat16")
    fp8 = _np_dt(mybir, "float8e4")

    # e_t[g, r, s, n] = e_sorted[n, (2g+s)*128 + r]
    e_t = np.ascontiguousarray(
        e_sorted.T.reshape(NG, 2, 128, N).transpose(0, 2, 1, 3)).astype(fp8)

    xw = x * w[None, :]                               # (B, D)
    x2w = (x * x) @ w                                 # (B,)
    e2w = (e_sorted * e_sorted) @ w                   # (N,)

    aug = np.zeros((1, N + 128), dtype=bf16)
    aug[0, :N] = (-0.5 * S_SCALE * e2w).astype(bf16)
    aug[0, N:] = np.ones(128, dtype=bf16)

    per_core = []
    for cid in range(NCORES):
        rows = slice(cid * B_LOC, (cid + 1) * B_LOC)
        xw_c = S_SCALE * xw[rows]                     # (128, 512)
        # xw_t[r, g, s, m] = S * xw_c[m, (2g+s)*128+r]
        xw_t = np.ascontiguousarray(
            xw_c.T.reshape(NG, 2, 128, B_LOC).transpose(2, 0, 1, 3)).astype(fp8)
        bias_c = (-beta * x2w[rows]).astype(np.float32).reshape(B_LOC, 1)
        per_core.append({
            "e_t": e_t,
            "xw_t": xw_t,
            "aug": aug,
            "bias": bias_c,
        })
    return per_core, counts, beta, gamma


def kernel(x, ex_feats, ex_labels, w_unconstrained, gamma_unconstrained,
           beta_unconstrained, _want_results=False, **run_kwargs):
    from concourse.bass_utils import run_bass_kernel_spmd

    per_core, counts, beta, gamma = _prepare(
        x, ex_feats, ex_labels, w_unconstrained, gamma_unconstrained,
        beta_unconstrained)

    pieces, maxp = _compute_pieces(counts)
    key = (tuple(pieces), maxp, round(beta, 12), round(gamma, 12))
    if key not in _prog_cache:
        _prog_cache[key] = _build_program(pieces, maxp, beta, gamma)
    nc = _prog_cache[key]

    res = run_bass_kernel_spmd(nc, per_core, list(range(NCORES)), **run_kwargs)
    out = np.concatenate(
        [np.asarray(res.results[cid]["logits"], dtype=np.float32)
         for cid in range(NCORES)], axis=0)
    if _want_results:
        return out, res
    return out



# revision 2
# speedup vs baseline: 1.4821x; 1.4821x over previous
"""ExemplarAttention Trainium2 kernel (8 NeuronCores, exemplar-sharded).

logits[b,c] = gamma * log(sum_{n:label[n]=c} exp(-beta * sum_k w_k (x[b,k]-e[n,k])^2) + eps)

Strategy (v2 — exemplar/N-sharded, transposed GEMM):
  - Shard the N=16384 exemplars across the 8 cores (~2048 each) and
    replicate the batch. Per-core DMA drops from 8MB (replicated bank)
    to ~1.6MB, and the per-class scatter-add becomes a host-side gather.
  - Transposed GEMM orientation: psum[n_part, b_free] = S * cross with
    exemplars on psum PARTITIONS (stationary = exemplar features, moving
    = S*x*w, both fp8 DoubleRow, K=2x256). The per-exemplar -beta*e2w
    term is now a per-partition constant -> it rides the ScalarE
    activation BIAS, eliminating the baseline's whole aug-matmul pass.
  - Exemplars are packed so each partition holds NT=17 same-class
    exemplars (one "chunk"), sorted by e2w. Tile pairs then share one
    bias value (pair-mean of e2w; adjacent-in-sorted-order so the
    approximation error is ~1e-4 relative), letting ScalarE exp whole
    2-tile [128,2048] psum chunks in one instruction.
  - Per-class reduction = elementwise bf16 adds over the 17 tiles on the
    otherwise-idle VectorE (2x mode), into 3 accumulators that stream
    out early. Host: sum partitions by class, apply exp(-beta*x2w[b]),
    gamma*log(.+eps) in f64.
  - ScalarE's exp (~2.2M elements/core @ 1 elem/lane/cycle) is the
    irreducible bottleneck (~15-17us); PE (~14.5us@1.2GHz), DVE (~11us)
    and DMA (~5us) all hide under it.
"""

import os
from contextlib import ExitStack

import numpy as np

B, N, D, C = 1024, 16384, 512, 10
NCORES = 8
NT = 17                      # exemplar slots per partition (one class chunk)
NG = 2                       # DoubleRow K-groups (K=256 each)
NTILE = 512                  # matmul free dim (1 psum bank)
EPS = 1e-9
S_SCALE = 128.0              # fp8 scale applied to x*w
PAD_BIAS = -100.0            # bias for all-padding chunks: exp(-100) == 0

# psum chunks: single tile first (fast pipeline start), then 8 pairs.
CHUNKS = [(0,)] + [(2 * j + 1, 2 * j + 2) for j in range(8)]
# acc group per tile: tiles 0-4 -> acc0, 5-10 -> acc1, 11-16 -> acc2
ACC_LAST_CHUNK = {0: 2, 1: 5, 2: 8}  # acc idx -> chunk after which it's final


def _acc_of_tile(t):
    return 0 if t < 5 else (1 if t < 11 else 2)


N_WARMUP_MM = 6

_prog_cache = {}


def _np_dt(mybir, name):
    return mybir.dt.np(getattr(mybir.dt, name))


def _build_program(act_scale):
    import concourse.bass as bass  # noqa: F401
    import concourse.tile as tile
    from concourse import bacc, mybir

    fp8 = mybir.dt.float8e4
    bf16 = mybir.dt.bfloat16
    f32 = mybir.dt.float32
    DR = mybir.MatmulPerfMode.DoubleRow
    ADD = mybir.AluOpType.add

    nc = bacc.Bacc("TRN2", target_bir_lowering=False, debug=False,
                   num_devices=NCORES)

    # DRAM layouts mirror the SBUF layouts (partition-major) so each load
    # is a plain strided DMA.
    e_d = nc.dram_tensor("e_t", [128, NT, NG, 2, 128], fp8,
                         kind="ExternalInput").ap()
    xw_d = nc.dram_tensor("xw_t", [128, NG, 2, B], fp8,
                          kind="ExternalInput").ap()
    bias_d = nc.dram_tensor("bias", [128, len(CHUNKS)], f32,
                            kind="ExternalInput").ap()
    out_d = nc.dram_tensor("acc", [3, 128, B], bf16, kind="ExternalOutput").ap()

    with tile.TileContext(nc) as tc, ExitStack() as ctx:
        singles = ctx.enter_context(tc.tile_pool(name="singles", bufs=1))
        psum_pool = ctx.enter_context(tc.tile_pool(name="ps", bufs=2,
                                                   space="PSUM"))
        tmp_pool = ctx.enter_context(tc.tile_pool(name="tmp", bufs=3))

        xw_sb = singles.tile([128, NG, 2, B], fp8)
        e_sb = singles.tile([128, NT, NG, 2, 128], fp8)
        bias_sb = singles.tile([128, len(CHUNKS)], f32)
        accs = [singles.tile([128, B], bf16, name=f"acc{i}") for i in range(3)]

        # Warmup matmul operands: memset tile, no DMA dependency.
        dmy = singles.tile([128, 2, NTILE + 128], fp8)
        nc.vector.memset(dmy[:, :, :], 0.0)

        # Input DMA: two rings (SP sync + Pool/gpsimd), nothing on the
        # Act/DVE sequencers (dma_start costs ~667ns of sequencer time
        # there, and Act is the bottleneck engine).
        nc.sync.dma_start(out=xw_sb[:, 0, :, 0:NTILE],
                          in_=xw_d[:, 0, :, 0:NTILE])
        nc.gpsimd.dma_start(out=bias_sb[:, :], in_=bias_d[:, :])
        nc.gpsimd.dma_start(out=xw_sb[:, 1, :, :], in_=xw_d[:, 1, :, :])
        nc.sync.dma_start(out=xw_sb[:, 0, :, NTILE:B],
                          in_=xw_d[:, 0, :, NTILE:B])
        for j, tiles_ in enumerate(CHUNKS):
            ring = [nc.sync, nc.gpsimd][j % 2]
            t0, t1 = tiles_[0], tiles_[-1] + 1
            ring.dma_start(out=e_sb[:, t0:t1, :, :, :],
                           in_=e_d[:, t0:t1, :, :, :])

        # Warmup: ramp the PE clock + let the DMA stream get ahead.
        ps0 = psum_pool.tile([128, 2048], f32, tag="ps", name="ps0")
        for _ in range(N_WARMUP_MM):
            nc.tensor.matmul(ps0[:, 0:NTILE], lhsT=dmy[:, :, NTILE:NTILE + 128],
                             rhs=dmy[:, :, 0:NTILE], start=True, stop=True,
                             perf_mode=DR)

        acc_touched = [False, False, False]
        for j, tiles_ in enumerate(CHUNKS):
            ps = ps0 if j == 0 else psum_pool.tile([128, 2048], f32, tag="ps",
                                                   name=f"ps{j}")
            for ti, t in enumerate(tiles_):
                for g in range(NG):
                    for h in range(2):
                        c0 = ti * 1024 + h * NTILE
                        nc.tensor.matmul(
                            ps[:, c0:c0 + NTILE],
                            lhsT=e_sb[:, t, g, :, :],
                            rhs=xw_sb[:, g, :, h * NTILE:(h + 1) * NTILE],
                            start=(g == 0), stop=(g == NG - 1),
                            perf_mode=DR)

            w = 1024 * len(tiles_)
            tmp = tmp_pool.tile([128, 2048], bf16, tag="tmp")
            nc.scalar.activation(
                out=tmp[:, 0:w],
                in_=ps[:, 0:w],
                func=mybir.ActivationFunctionType.Exp,
                bias=bias_sb[:, j:j + 1],
                scale=act_scale,
            )
            for ti, t in enumerate(tiles_):
                a = accs[_acc_of_tile(t)]
                sl = tmp[:, ti * 1024:(ti + 1) * 1024]
                if not acc_touched[_acc_of_tile(t)]:
                    acc_touched[_acc_of_tile(t)] = True
                    nc.vector.tensor_scalar_mul(a[:, :], sl, 1.0)
                else:
                    nc.vector.tensor_tensor(out=a[:, :], in0=a[:, :], in1=sl,
                                            op=ADD)
            for ai, jlast in ACC_LAST_CHUNK.items():
                if j == jlast:
                    nc.gpsimd.dma_start(out=out_d[ai, :, :], in_=accs[ai][:, :])

    nc.compile()
    return nc


def _pack(labels, e2w, beta):
    """Pack exemplars into per-core [128 partition, NT slot] grids.

    Each partition holds <=NT exemplars of ONE class, consecutive in
    e2w-sorted order (so Act-chunk pair-mean biases are accurate).
    Returns per-core (grid_idx [128,NT] int64 (-1 pad), bias [128,9] f32,
    pad_const [128] f64, cls_of_part [128] int64 (-1 unused)).
    """
    chunks = []  # (class, np.array of exemplar ids, e2w-sorted)
    for c in range(C):
        idx = np.where(labels == c)[0]
        idx = idx[np.argsort(e2w[idx], kind="stable")]
        for s in range(0, len(idx), NT):
            chunks.append((c, idx[s:s + NT]))

    per_core = [[] for _ in range(NCORES)]
    for k, ch in enumerate(chunks):
        per_core[k % NCORES].append(ch)
    assert max(len(p) for p in per_core) <= 128, \
        f"chunk packing overflow: {[len(p) for p in per_core]}"

    out = []
    bf16 = None
    for cid in range(NCORES):
        grid = np.full((128, NT), -1, dtype=np.int64)
        cls = np.full(128, -1, dtype=np.int64)
        for p, (c, ids) in enumerate(per_core[cid]):
            grid[p, :len(ids)] = ids
            cls[p] = c
        # biases per chunk (pair-mean of -beta*e2w over real slots)
        bias = np.full((128, len(CHUNKS)), PAD_BIAS, dtype=np.float64)
        npad = np.zeros((128, len(CHUNKS)), dtype=np.int64)
        e2w_g = np.where(grid >= 0, e2w[grid.clip(0)], np.nan)
        for j, tiles_ in enumerate(CHUNKS):
            vals = e2w_g[:, list(tiles_)]
            cnt = np.sum(~np.isnan(vals), axis=1)
            m = cnt > 0
            bias[m, j] = -beta * np.nanmean(vals[m], axis=1)
            npad[:, j] = np.where(m, len(tiles_) - cnt, 0)
        bias_f32 = bias.astype(np.float32)
        # padding slots in half-real chunks contribute exp(bias) per pad
        # (their psum column is exactly 0); subtract on host. Round
        # through bf16 to match the device's Act output dtype.
        import concourse.mybir as mybir
        if bf16 is None:
            bf16 = _np_dt(mybir, "bfloat16")
        pad_term = np.exp(bias_f32.astype(np.float64))
        pad_term = pad_term.astype(bf16).astype(np.float64)
        pad_const = np.sum(npad * pad_term, axis=1)
        out.append((grid, bias_f32, pad_const, cls))
    return out


def _prepare(x, ex_feats, ex_labels, w_unconstrained, gamma_unconstrained,
             beta_unconstrained):
    from concourse import mybir

    x = np.asarray(x, dtype=np.float64)
    e = np.asarray(ex_feats, dtype=np.float64)
    labels = np.asarray(ex_labels).astype(np.int64)
    wu = np.asarray(w_unconstrained, dtype=np.float64)

    beta = float(np.log1p(np.exp(np.float64(beta_unconstrained)))) + EPS
    gamma = float(np.log1p(np.exp(np.float64(gamma_unconstrained)))) + EPS
    wexp = np.exp(wu - wu.max())
    w = wexp / wexp.sum() + EPS

    fp8 = _np_dt(mybir, "float8e4")

    x2w = (x * x) @ w                                 # (B,)
    e2w = (e * e) @ w                                 # (N,)
    e8 = np.ascontiguousarray(e.astype(fp8))          # (N, D)

    # xw_t[r, g, s, b] = S * x[b, g*256+s*128+r] * w[...]  (replicated)
    xw = (S_SCALE * (x * w[None, :])).astype(np.float32)
    xw_t = np.ascontiguousarray(
        xw.reshape(B, NG, 2, 128).transpose(3, 1, 2, 0)).astype(fp8)

    packs = _pack(labels, e2w, beta)
    per_core = []
    for cid in range(NCORES):
        grid, bias_f32, pad_const, cls = packs[cid]
        gf8 = e8[grid.clip(0)]                        # (128, NT, D)
        gf8[grid < 0] = fp8(0.0)
        # e_t[r, t, g, s, p] = gf8[p, t, g*256+s*128+r]
        e_t = np.ascontiguousarray(
            gf8.reshape(128, NT, NG, 2, 128).transpose(4, 1, 2, 3, 0))
        per_core.append({"e_t": e_t, "xw_t": xw_t, "bias": bias_f32})
    return per_core, packs, x2w, beta, gamma


def kernel(x, ex_feats, ex_labels, w_unconstrained, gamma_unconstrained,
           beta_unconstrained, _want_results=False, **run_kwargs):
    from concourse.bass_utils import run_bass_kernel_spmd

    per_core, packs, x2w, beta, gamma = _prepare(
        x, ex_feats, ex_labels, w_unconstrained, gamma_unconstrained,
        beta_unconstrained)

    act_scale = float(2.0 * beta / S_SCALE)
    key = round(act_scale, 12)
    if key not in _prog_cache:
        _prog_cache[key] = _build_program(act_scale)
    nc = _prog_cache[key]

    res = run_bass_kernel_spmd(nc, per_core, list(range(NCORES)), **run_kwargs)

    class_sum = np.zeros((B, C), dtype=np.float64)
    for cid in range(NCORES):
        acc = np.asarray(res.results[cid]["acc"]).astype(np.float64)  # (3,128,B)
        part = acc.sum(axis=0)                        # (128, B)
        grid, bias_f32, pad_const, cls = packs[cid]
        part -= pad_const[:, None]
        for c in range(C):
            m = cls == c
            if m.any():
                class_sum[:, c] += part[m].sum(axis=0)

    class_sum *= np.exp(-beta * x2w)[:, None]
    logits = (gamma * np.log(class_sum + EPS)).astype(np.float32)
    if _want_results:
        return logits, res
    return logits


# revision 6
# speedup vs baseline: 1.5288x; 1.0315x over previous
"""ExemplarAttention Trainium2 kernel (8 NeuronCores, exemplar-sharded).

logits[b,c] = gamma * log(sum_{n:label[n]=c} exp(-beta * sum_k w_k (x[b,k]-e[n,k])^2) + eps)

Strategy (v2 — exemplar/N-sharded, transposed GEMM):
  - Shard the N=16384 exemplars across the 8 cores (~2048 each) and
    replicate the batch. Per-core DMA drops from 8MB (replicated bank)
    to ~1.6MB, and the per-class scatter-add becomes a host-side gather.
  - Transposed GEMM orientation: psum[n_part, b_free] = S * cross with
    exemplars on psum PARTITIONS (stationary = exemplar features, moving
    = S*x*w, both fp8 DoubleRow, K=2x256). The per-exemplar -beta*e2w
    term is now a per-partition constant -> it rides the ScalarE
    activation BIAS, eliminating the baseline's whole aug-matmul pass.
  - Exemplars are packed so each partition holds NT=17 same-class
    exemplars (one "chunk"), sorted by e2w. Tile pairs then share one
    bias value (pair-mean of e2w; adjacent-in-sorted-order so the
    approximation error is ~1e-4 relative), letting ScalarE exp whole
    2-tile [128,2048] psum chunks in one instruction.
  - Per-class reduction = elementwise bf16 adds over the 17 tiles on the
    otherwise-idle VectorE (2x mode), into 3 accumulators that stream
    out early. Host: sum partitions by class, apply exp(-beta*x2w[b]),
    gamma*log(.+eps) in f64.
  - ScalarE's exp (~2.2M elements/core @ 1 elem/lane/cycle) is the
    irreducible bottleneck (~15-17us); PE (~14.5us@1.2GHz), DVE (~11us)
    and DMA (~5us) all hide under it.
"""

import os
from contextlib import ExitStack

import numpy as np

B, N, D, C = 1024, 16384, 512, 10
NCORES = 8
NT = 17                      # exemplar slots per partition (one class chunk)
NG = 2                       # DoubleRow K-groups (K=256 each)
NTILE = 512                  # matmul free dim (1 psum bank)
EPS = 1e-9
S_SCALE = 128.0              # fp8 scale applied to x*w
PAD_BIAS = -100.0            # bias for all-padding chunks: exp(-100) == 0

# psum chunks: singles at both ends (small first Act starts the pipeline
# early; small last Act + one tail DVE add shrink the tail), pairs between.
CHUNKS = ([(0,)] + [(2 * j + 1, 2 * j + 2) for j in range(7)]
          + [(15,), (16,)])
# acc group per tile: tiles 0-4 -> acc0, 5-10 -> acc1, 11-16 -> acc2
ACC_LAST_CHUNK = {0: 2, 1: 5, 2: 9}  # acc idx -> chunk after which it's final


def _acc_of_tile(t):
    return 0 if t < 5 else (1 if t < 11 else 2)


N_WARMUP_MM = 8

_prog_cache = {}


def _np_dt(mybir, name):
    return mybir.dt.np(getattr(mybir.dt, name))


def _build_program(act_scale):
    import concourse.bass as bass  # noqa: F401
    import concourse.tile as tile
    from concourse import bacc, mybir

    fp8 = mybir.dt.float8e4
    bf16 = mybir.dt.bfloat16
    f32 = mybir.dt.float32
    DR = mybir.MatmulPerfMode.DoubleRow
    ADD = mybir.AluOpType.add

    nc = bacc.Bacc("TRN2", target_bir_lowering=False, debug=False,
                   num_devices=NCORES)

    # DRAM layouts mirror the SBUF layouts (partition-major) so each load
    # is a plain strided DMA.
    e_d = nc.dram_tensor("e_t", [128, NT, NG, 2, 128], fp8,
                         kind="ExternalInput").ap()
    xw_d = nc.dram_tensor("xw_t", [128, NG, 2, B], fp8,
                          kind="ExternalInput").ap()
    bias_d = nc.dram_tensor("bias", [128, len(CHUNKS)], f32,
                            kind="ExternalInput").ap()
    out_d = nc.dram_tensor("acc", [3, 128, B], bf16, kind="ExternalOutput").ap()

    with tile.TileContext(nc) as tc, ExitStack() as ctx:
        singles = ctx.enter_context(tc.tile_pool(name="singles", bufs=1))
        psum_pool = ctx.enter_context(tc.tile_pool(name="ps", bufs=2,
                                                   space="PSUM"))
        tmp_pool = ctx.enter_context(tc.tile_pool(name="tmp", bufs=3))

        xw_sb = singles.tile([128, NG, 2, B], fp8)
        e_sb = singles.tile([128, NT, NG, 2, 128], fp8)
        bias_sb = singles.tile([128, len(CHUNKS)], f32)
        accs = [singles.tile([128, B], bf16, name=f"acc{i}") for i in range(3)]

        # Warmup matmul operands: small memset tile, no DMA dependency.
        dmy = singles.tile([128, 2, 256], fp8)
        nc.vector.memset(dmy[:, :, :], 0.0)

        # Input DMA. Each dma_start costs ~620ns of issuing-sequencer time
        # AND ~128 queue descriptors (~170ns each, one per partition row,
        # regardless of row bytes) -- so few, fat transfers win. Rings:
        # sync + gpsimd for inputs, vector (idle at the head) takes the
        # tiny bias, the Act sequencer carries nothing (it's the
        # bottleneck engine).
        nc.sync.dma_start(out=xw_sb[:, :, :, :], in_=xw_d[:, :, :, :])
        nc.gpsimd.dma_start(out=e_sb[:, 0:3, :, :, :],
                            in_=e_d[:, 0:3, :, :, :])
        nc.gpsimd.dma_start(out=e_sb[:, 3:9, :, :, :],
                            in_=e_d[:, 3:9, :, :, :])
        nc.gpsimd.dma_start(out=e_sb[:, 9:NT, :, :, :],
                            in_=e_d[:, 9:NT, :, :, :])
        nc.sync.dma_start(out=bias_sb[:, :], in_=bias_d[:, :])

        # Warmup: ramp the PE clock + let the DMA stream get ahead.
        ps0 = psum_pool.tile([128, 2048], f32, tag="ps", name="ps0")
        for _ in range(N_WARMUP_MM):
            nc.tensor.matmul(ps0[:, 0:256], lhsT=dmy[:, :, 0:128],
                             rhs=dmy[:, :, :], start=True, stop=True,
                             perf_mode=DR)

        acc_touched = [False, False, False]
        for j, tiles_ in enumerate(CHUNKS):
            ps = ps0 if j == 0 else psum_pool.tile([128, 2048], f32, tag="ps",
                                                   name=f"ps{j}")
            for ti, t in enumerate(tiles_):
                for g in range(NG):
                    for h in range(2):
                        c0 = ti * 1024 + h * NTILE
                        nc.tensor.matmul(
                            ps[:, c0:c0 + NTILE],
                            lhsT=e_sb[:, t, g, :, :],
                            rhs=xw_sb[:, g, :, h * NTILE:(h + 1) * NTILE],
                            start=(g == 0), stop=(g == NG - 1),
                            perf_mode=DR)

            w = 1024 * len(tiles_)
            tmp = tmp_pool.tile([128, 2048], bf16, tag="tmp")
            nc.scalar.activation(
                out=tmp[:, 0:w],
                in_=ps[:, 0:w],
                func=mybir.ActivationFunctionType.Exp,
                bias=bias_sb[:, j:j + 1],
                scale=act_scale,
            )
            for ti, t in enumerate(tiles_):
                a = accs[_acc_of_tile(t)]
                sl = tmp[:, ti * 1024:(ti + 1) * 1024]
                if not acc_touched[_acc_of_tile(t)]:
                    acc_touched[_acc_of_tile(t)] = True
                    nc.vector.tensor_scalar_mul(a[:, :], sl, 1.0)
                else:
                    nc.vector.tensor_tensor(out=a[:, :], in0=a[:, :], in1=sl,
                                            op=ADD)
            for ai, jlast in ACC_LAST_CHUNK.items():
                if j == jlast:
                    nc.gpsimd.dma_start(out=out_d[ai, :, :], in_=accs[ai][:, :])

    nc.compile()
    return nc


def _pack(labels, e2w, beta):
    """Pack exemplars into per-core [128 partition, NT slot] grids.

    Each partition holds <=NT exemplars of ONE class, consecutive in
    e2w-sorted order (so Act-chunk pair-mean biases are accurate).
    Returns per-core (grid_idx [128,NT] int64 (-1 pad), bias [128,9] f32,
    pad_const [128] f64, cls_of_part [128] int64 (-1 unused)).
    """
    chunks = []  # (class, np.array of exemplar ids, e2w-sorted)
    for c in range(C):
        idx = np.where(labels == c)[0]
        idx = idx[np.argsort(e2w[idx], kind="stable")]
        for s in range(0, len(idx), NT):
            chunks.append((c, idx[s:s + NT]))

    per_core = [[] for _ in range(NCORES)]
    for k, ch in enumerate(chunks):
        per_core[k % NCORES].append(ch)
    assert max(len(p) for p in per_core) <= 128, \
        f"chunk packing overflow: {[len(p) for p in per_core]}"

    out = []
    bf16 = None
    for cid in range(NCORES):
        grid = np.full((128, NT), -1, dtype=np.int64)
        cls = np.full(128, -1, dtype=np.int64)
        for p, (c, ids) in enumerate(per_core[cid]):
            grid[p, :len(ids)] = ids
            cls[p] = c
        # biases per chunk (pair-mean of -beta*e2w over real slots)
        bias = np.full((128, len(CHUNKS)), PAD_BIAS, dtype=np.float64)
        npad = np.zeros((128, len(CHUNKS)), dtype=np.int64)
        e2w_g = np.where(grid >= 0, e2w[grid.clip(0)], np.nan)
        for j, tiles_ in enumerate(CHUNKS):
            vals = e2w_g[:, list(tiles_)]
            cnt = np.sum(~np.isnan(vals), axis=1)
            m = cnt > 0
            bias[m, j] = -beta * np.nanmean(vals[m], axis=1)
            npad[:, j] = np.where(m, len(tiles_) - cnt, 0)
        bias_f32 = bias.astype(np.float32)
        # padding slots in half-real chunks contribute exp(bias) per pad
        # (their psum column is exactly 0); subtract on host. Round
        # through bf16 to match the device's Act output dtype.
        import concourse.mybir as mybir
        if bf16 is None:
            bf16 = _np_dt(mybir, "bfloat16")
        pad_term = np.exp(bias_f32.astype(np.float64))
        pad_term = pad_term.astype(bf16).astype(np.float64)
        pad_const = np.sum(npad * pad_term, axis=1)
        out.append((grid, bias_f32, pad_const, cls))
    return out


def _prepare(x, ex_feats, ex_labels, w_unconstrained, gamma_unconstrained,
             beta_unconstrained):
    from concourse import mybir

    x = np.asarray(x, dtype=np.float64)
    e = np.asarray(ex_feats, dtype=np.float64)
    labels = np.asarray(ex_labels).astype(np.int64)
    wu = np.asarray(w_unconstrained, dtype=np.float64)

    beta = float(np.log1p(np.exp(np.float64(beta_unconstrained)))) + EPS
    gamma = float(np.log1p(np.exp(np.float64(gamma_unconstrained)))) + EPS
    wexp = np.exp(wu - wu.max())
    w = wexp / wexp.sum() + EPS

    fp8 = _np_dt(mybir, "float8e4")

    x2w = (x * x) @ w                                 # (B,)
    e2w = (e * e) @ w                                 # (N,)
    e8 = np.ascontiguousarray(e.astype(fp8))          # (N, D)

    # xw_t[r, g, s, b] = S * x[b, g*256+s*128+r] * w[...]  (replicated)
    xw = (S_SCALE * (x * w[None, :])).astype(np.float32)
    xw_t = np.ascontiguousarray(
        xw.reshape(B, NG, 2, 128).transpose(3, 1, 2, 0)).astype(fp8)

    packs = _pack(labels, e2w, beta)
    per_core = []
    for cid in range(NCORES):
        grid, bias_f32, pad_const, cls = packs[cid]
        gf8 = e8[grid.clip(0)]                        # (128, NT, D)
        gf8[grid < 0] = fp8(0.0)
        # e_t[r, t, g, s, p] = gf8[p, t, g*256+s*128+r]
        e_t = np.ascontiguousarray(
            gf8.reshape(128, NT, NG, 2, 128).transpose(4, 1, 2, 3, 0))
        per_core.append({"e_t": e_t, "xw_t": xw_t, "bias": bias_f32})
    return per_core, packs, x2w, beta, gamma


def kernel(x, ex_feats, ex_labels, w_unconstrained, gamma_unconstrained,
           beta_unconstrained, _want_results=False, **run_kwargs):
    from concourse.bass_utils import run_bass_kernel_spmd

    per_core, packs, x2w, beta, gamma = _prepare(
        x, ex_feats, ex_labels, w_unconstrained, gamma_unconstrained,
        beta_unconstrained)

    act_scale = float(2.0 * beta / S_SCALE)
    key = round(act_scale, 12)
    if key not in _prog_cache:
        _prog_cache[key] = _build_program(act_scale)
    nc = _prog_cache[key]

    res = run_bass_kernel_spmd(nc, per_core, list(range(NCORES)), **run_kwargs)

    class_sum = np.zeros((B, C), dtype=np.float64)
    for cid in range(NCORES):
        acc = np.asarray(res.results[cid]["acc"]).astype(np.float64)  # (3,128,B)
        part = acc.sum(axis=0)                        # (128, B)
        grid, bias_f32, pad_const, cls = packs[cid]
        part -= pad_const[:, None]
        for c in range(C):
            m = cls == c
            if m.any():
                class_sum[:, c] += part[m].sum(axis=0)

    class_sum *= np.exp(-beta * x2w)[:, None]
    logits = (gamma * np.log(class_sum + EPS)).astype(np.float32)
    if _want_results:
        return logits, res
    return logits
